# revision 1
# baseline (speedup 1.0000x reference)
"""Trainium2 Bass kernel for nn_DispersiveLoss (B=2048, D=16*768=12288, 8 cores).

Strategy (circulant block decomposition, uniform SPMD):
  x (2048, 12288) -> 16 row-blocks of 128. Core c "owns" m-blocks {2c, 2c+1}
  and computes two Gram strips G[m, m..m+8 (mod 16)] (width 9 blocks = 1152)
  in bf16 with D on partitions (96 k-chunks of 128, PSUM fp32 accumulation).
  Every unordered block pair lands exactly once (circular distance 1..7),
  diagonal blocks are masked to the upper triangle, distance-8 blocks are
  computed twice and weighted 0.5 -- all via one shared mask + ACT scales,
  so the program is identical on all 8 cores (pure SPMD).

  Launch A computes row norms sq_i = ||x_i||^2 for each core's own 256 rows
  (natural layout, DVE/ACT split). The host gathers/redistributes sq (pure
  data movement), then launch B consumes the Gram strips:
    u = d2 - 2D = -2*(g - (sq_n - 2D)/2) + sq_m
  where the per-column term is folded into PSUM by a K=1 ones-matmul and the
  per-partition term rides the ACT bias. ACT produces exp(-u/(D*tau)), u, u^2
  with per-instruction accum_out reductions; DVE handles the triangle-masked
  diagonal blocks. Host combines per-core (128,4) partial sums in float64.
"""

import os

import numpy as np
import ml_dtypes

import concourse.bass as bass
import concourse.mybir as mybir
import concourse.tile as tile
from concourse import bacc
from concourse.bass_utils import run_bass_kernel_spmd

NC_N = 8
B, D = 2048, 12288
BLK = 128
UNION = 1280  # 10 blocks per core in SBUF
STRIPW = 1152  # 9-block strip width
KCH = 96  # k-chunks of 128
KB = 8  # k-chunks per DMA batch
TAU = 0.5
CC = float(2 * D)  # centering constant (E[d2] for N(0,1) rows)
SS = 1.0 / (D * TAU)  # exponent scale
F32 = mybir.dt.float32
BF16 = mybir.dt.bfloat16
USE_FP8 = os.environ.get("KERNEL_NO_FP8", "") == ""
if USE_FP8:
    DT_IN = mybir.dt.float8e4
    NP_IN = ml_dtypes.float8_e4m3
else:
    DT_IN = BF16
    NP_IN = ml_dtypes.bfloat16
LN_HALF = float(np.log(0.5))
INV_SQRT2 = float(1.0 / np.sqrt(2.0))

# effective pair count: 16 * (tri 8128 + 7 full blocks + half block)
N_PAIRS = B * (B - 1) // 2

KERNEL_EXEC_NS = []  # filled when KERNEL_TRACE is set (test harness only)

_cache = {}


def _trace_enabled():
    return bool(os.environ.get("KERNEL_TRACE"))


def _build_sq_kernel():
    """Launch A: per core, sq for its own 256 rows from natural-layout bf16."""
    nc = bacc.Bacc("TRN2", target_bir_lowering=False, debug=False, num_devices=NC_N)
    xn = nc.dram_tensor("xn", [2, BLK, D], DT_IN, kind="ExternalInput")
    sq_out = nc.dram_tensor("sq_out", [2, BLK], F32, kind="ExternalOutput")
    HD = D // 2  # half-block DMA granularity
    NSUB = 4  # compute sub-slices per half

    with tile.TileContext(nc) as tc:
        with (
            tc.tile_pool(name="p", bufs=4) as p,
            tc.tile_pool(name="a", bufs=1) as a,
        ):
            acc = a.tile([BLK, 16], F32)
            # 4 big DMAs (block-half each); DVE handles block 0, ACT block 1
            SUB = HD // NSUB
            for h in range(2):
                for b in range(2):
                    t = p.tile([BLK, HD], DT_IN, tag="xin")
                    nc.sync.dma_start(t[:], xn[b, :, h * HD : (h + 1) * HD])
                    for j in range(NSUB):
                        col_i = b * 8 + h * NSUB + j
                        col = acc[:, col_i : col_i + 1]
                        ts_ = t[:, j * SUB : (j + 1) * SUB]
                        scr = p.tile([BLK, SUB], F32, tag="scr")
                        if b == 0:
                            nc.vector.scalar_tensor_tensor(
                                out=scr[:],
                                in0=ts_,
                                scalar=1.0,
                                in1=ts_,
                                op0=mybir.AluOpType.mult,
                                op1=mybir.AluOpType.mult,
                                accum_out=col,
                            )
                        else:
                            nc.scalar.activation(
                                scr[:],
                                ts_,
                                mybir.ActivationFunctionType.Square,
                                accum_out=col,
                            )
            r = a.tile([BLK, 2], F32)
            nc.vector.tensor_reduce(
                r[:, 0:1], acc[:, 0:8], mybir.AxisListType.X, mybir.AluOpType.add
            )
            nc.vector.tensor_reduce(
                r[:, 1:2], acc[:, 8:16], mybir.AxisListType.X, mybir.AluOpType.add
            )
            for b in range(2):
                nc.sync.dma_start(
                    sq_out[b].rearrange("(p o) -> p o", o=1), r[:, b : b + 1]
                )
    nc.compile()
    return nc


def _build_main_kernel():
    """Launch B: Gram strips + statistics."""
    nc = bacc.Bacc("TRN2", target_bir_lowering=False, debug=False, num_devices=NC_N)
    xT = nc.dram_tensor("xT", [BLK, KCH, UNION], DT_IN, kind="ExternalInput")
    sqw = nc.dram_tensor("sqw", [UNION], F32, kind="ExternalInput")
    tri = nc.dram_tensor("tri", [BLK, BLK], F32, kind="ExternalInput")
    out_stats = nc.dram_tensor("out_stats", [BLK, 4], F32, kind="ExternalOutput")

    MULT = mybir.AluOpType.mult
    ADD = mybir.AluOpType.add
    EXP = mybir.ActivationFunctionType.Exp
    SQUARE = mybir.ActivationFunctionType.Square
    IDENT = mybir.ActivationFunctionType.Identity
    SQRT = mybir.ActivationFunctionType.Sqrt

    # acc columns: 0:6 E [mid1, mid2, half1, half2, tri1, tri2]
    #              6:12 S1, 12:18 S2, 18 feat
    with tile.TileContext(nc) as tc:
        with (
            tc.tile_pool(name="slab", bufs=4) as slab_pool,
            tc.tile_pool(name="psp", bufs=1, space="PSUM") as psp,
            tc.tile_pool(name="post", bufs=2) as post,
            tc.tile_pool(name="accp", bufs=1) as accp,
        ):
            ps0 = psp.tile([BLK, STRIPW], F32, tag="ps0")
            ps1 = psp.tile([BLK, STRIPW], F32, tag="ps1")
            ps = [ps0, ps1]
            acc = accp.tile([BLK, 19], F32)

            # --- early work that only depends on inputs ---
            # PE pre-warm: keep the HAM activity window busy while slab 0 DMAs
            warm = post.tile([BLK, 512], DT_IN, tag="warm")
            nc.gpsimd.memset(warm[:], 0.0)
            wps = psp.tile([BLK, 512], F32, tag="wps")
            for _ in range(14):
                nc.tensor.matmul(
                    wps[:], warm[:, 0:128], warm[:], start=True, stop=True,
                    skip_group_check=True,
                )
            # feat partial: sum sqrt(sq_own) over own 256 rows (input-only dep)
            ft = post.tile([BLK, 2], F32, tag="ft")
            nc.sync.dma_start(ft[:], sqw[0:256].rearrange("(j p) -> p j", j=2))
            fscr = post.tile([BLK, 2], F32, tag="fscr")
            nc.scalar.activation(fscr[:], ft[:], SQRT, accum_out=acc[:, 18:19])
            # preload the Exp activation table AFTER the sqrt so it stays
            # resident for the tail exp ops
            zcol = post.tile([BLK, 1], F32, tag="zcol")
            nc.gpsimd.memset(zcol[:], 0.0)
            zscr = post.tile([BLK, 1], F32, tag="zscr")
            nc.scalar.activation(zscr[:], zcol[:], mybir.ActivationFunctionType.Exp)

            # strip s: m-block at union col 128*s, window = union cols 128*s..128*s+1152
            segs = [(0, 512), (512, 1024), (1024, 1152)]

            KSTEP = 2 if USE_FP8 else 1
            DR = mybir.MatmulPerfMode.DoubleRow if USE_FP8 else None
            for kb in range(KCH // KB):
                st = slab_pool.tile([BLK, KB, UNION], DT_IN, tag="slab")
                nc.sync.dma_start(st[:], xT[:, kb * KB : (kb + 1) * KB, :])
                for ii in range(0, KB, KSTEP):
                    k = kb * KB + ii
                    for s in range(2):
                        off = 128 * s
                        lhs = st[:, ii : ii + KSTEP, off : off + 128]
                        for c0, c1 in segs:
                            nc.tensor.matmul(
                                ps[s][:, c0:c1],
                                lhs,
                                st[:, ii : ii + KSTEP, off + c0 : off + c1],
                                start=(k == 0),
                                stop=False,
                                perf_mode=DR,
                            )

            # ---- post processing ----
            # Fold BOTH rank-1 terms into PSUM with K=1 matmuls so that
            #   p := ps = g - (sq_n - CC)/2 - sq_m/2  =  -(d2 - CC)/2 = -u/2.
            # Stats then read PSUM directly (host rescales: S1 = -2*sum p,
            # S2 = 4*sum p^2; exp(-S*u) = exp(2*S*p) on ACT).
            ones = post.tile([1, 512], F32, tag="ones")
            nc.gpsimd.memset(ones[:], 1.0)
            sqrow = post.tile([1, UNION], F32, tag="sqrow")
            nc.sync.dma_start(sqrow[:], sqw[:].rearrange("(a b) -> a b", a=1))
            vrow = post.tile([1, UNION], F32, tag="vrow")
            # v = (sq - CC) * (-0.5)
            nc.vector.tensor_scalar(
                out=vrow[:],
                in0=sqrow[:],
                scalar1=-CC,
                scalar2=-0.5,
                op0=ADD,
                op1=MULT,
            )
            wrow = post.tile([1, 256], F32, tag="wrow")
            # w = -sq_own/2 for the two m-blocks
            nc.vector.tensor_scalar(
                out=wrow[:], in0=sqrow[:, 0:256], scalar1=-0.5, scalar2=0.0,
                op0=MULT, op1=ADD,
            )
            # per-column: ps += ones^T (1x128) @ v  (shared lhsT)
            for s in range(2):
                off = 128 * s
                for c0, c1 in segs:
                    nc.tensor.matmul(
                        ps[s][:, c0:c1],
                        ones[:, 0:128],
                        vrow[:, off + c0 : off + c1],
                        start=False,
                        stop=False,
                    )
            # per-row: ps += w^T (1x128, = -sq_m/2) @ ones
            for s in range(2):
                for c0, c1 in segs:
                    nc.tensor.matmul(
                        ps[s][:, c0:c1],
                        wrow[:, 128 * s : 128 * s + 128],
                        ones[:, 0 : c1 - c0],
                        start=False,
                        stop=(c0, c1) == segs[-1],
                    )

            tri_t = post.tile([BLK, BLK], F32, tag="tri")
            nc.sync.dma_start(tri_t[:], tri[:])
            lnhalf = post.tile([BLK, 1], F32, tag="lnhalf")
            nc.gpsimd.memset(lnhalf[:], LN_HALF)

            S2E = 2.0 * SS
            for s in range(2):
                p = ps[s]
                # mid region (full-weight blocks): cols 128:1024
                pm = p[:, 128:1024]
                scr = post.tile([BLK, 896], F32, tag="scr")
                nc.scalar.activation(scr[:], pm, EXP, scale=S2E, accum_out=acc[:, s : s + 1])
                # copy PSUM->SBUF with fused S1 accumulation, then square
                # against the PSUM copy (DVE allows only one PSUM operand)
                pmS = post.tile([BLK, 896], F32, tag="pmS")
                nc.vector.tensor_scalar(
                    out=pmS[:], in0=pm, scalar1=1.0, scalar2=0.0,
                    op0=MULT, op1=ADD, accum_out=acc[:, 6 + s : 7 + s],
                )
                scrd = post.tile([BLK, 896], F32, tag="scrd")
                nc.vector.scalar_tensor_tensor(
                    out=scrd[:], in0=pmS[:], scalar=1.0, in1=pm,
                    op0=MULT, op1=MULT, accum_out=acc[:, 12 + s : 13 + s],
                )

                # half-weight region (distance-8 block, computed twice fleet-wide):
                # cols 1024:1152; weight 0.5 folded into scales
                ph = p[:, 1024:1152]
                scr2 = post.tile([BLK, BLK], F32, tag="scr2")
                nc.scalar.activation(
                    scr2[:], ph, EXP, bias=lnhalf[:], scale=S2E,
                    accum_out=acc[:, 2 + s : 3 + s],
                )
                phS = post.tile([BLK, BLK], F32, tag="phS")
                nc.vector.tensor_scalar(
                    out=phS[:], in0=ph, scalar1=0.5, scalar2=0.0,
                    op0=MULT, op1=ADD, accum_out=acc[:, 8 + s : 9 + s],
                )
                scr2d = post.tile([BLK, BLK], F32, tag="scr2d")
                nc.vector.scalar_tensor_tensor(
                    out=scr2d[:], in0=phS[:], scalar=1.0, in1=ph,
                    op0=MULT, op1=MULT, accum_out=acc[:, 14 + s : 15 + s],
                )

                # diagonal block (upper-triangle mask): cols 0:128
                pd = p[:, 0:128]
                et = post.tile([BLK, BLK], F32, tag=f"et{s}")
                nc.scalar.activation(et[:], pd, EXP, scale=S2E)
                me = post.tile([BLK, BLK], F32, tag="me")
                nc.vector.scalar_tensor_tensor(
                    out=me[:], in0=et[:], scalar=1.0, in1=tri_t[:],
                    op0=MULT, op1=MULT, accum_out=acc[:, 4 + s : 5 + s],
                )
                mu = post.tile([BLK, BLK], F32, tag=f"mu{s}")
                nc.vector.scalar_tensor_tensor(
                    out=mu[:], in0=pd, scalar=1.0, in1=tri_t[:],
                    op0=MULT, op1=MULT, accum_out=acc[:, 10 + s : 11 + s],
                )
                ms2 = post.tile([BLK, BLK], F32, tag="ms2")
                nc.vector.scalar_tensor_tensor(
                    out=ms2[:], in0=mu[:], scalar=1.0, in1=pd,
                    op0=MULT, op1=MULT, accum_out=acc[:, 16 + s : 17 + s],
                )

            outt = accp.tile([BLK, 4], F32)
            nc.vector.tensor_reduce(outt[:, 0:1], acc[:, 0:6], mybir.AxisListType.X, ADD)
            nc.vector.tensor_reduce(outt[:, 1:2], acc[:, 6:12], mybir.AxisListType.X, ADD)
            nc.vector.tensor_reduce(outt[:, 2:3], acc[:, 12:18], mybir.AxisListType.X, ADD)
            nc.vector.tensor_copy(outt[:, 3:4], acc[:, 18:19])
            nc.sync.dma_start(out_stats[:], outt[:])
    nc.compile()
    return nc


def _get(name, builder):
    if name not in _cache:
        _cache[name] = builder()
    return _cache[name]


def _run(nc, in_maps, tag):
    if _trace_enabled():
        try:
            import profhook

            profhook.install()
        except Exception:
            pass
        import tempfile

        res = run_bass_kernel_spmd(
            nc, in_maps, list(range(NC_N)), trace=True,
            tmpdir=tempfile.mkdtemp(prefix=f"ktrace_{tag}_"),
        )
        KERNEL_EXEC_NS.append((tag, res.exec_time_ns))
        return res.results
    return run_bass_kernel_spmd(nc, in_maps, list(range(NC_N))).results


def kernel(features):
    x = np.asarray(features).reshape(B, D)
    xbf = x.astype(NP_IN)

    # ---- launch A: row norms ----
    a_maps = [
        {"xn": np.ascontiguousarray(xbf[256 * c : 256 * c + 256]).reshape(2, BLK, D)}
        for c in range(NC_N)
    ]
    nc_a = _get("sq", _build_sq_kernel)
    a_res = _run(nc_a, a_maps, "sq")
    sq_full = np.concatenate([a_res[c]["sq_out"].reshape(256) for c in range(NC_N)])

    # ---- launch B: Gram strips + stats ----
    xT_full = np.ascontiguousarray(xbf.T)  # (D, B)
    b_maps = []
    tri = np.triu(np.ones((BLK, BLK), np.float32), k=1)
    for c in range(NC_N):
        cols = (256 * c + np.arange(UNION)) % B
        xu = xT_full[:, cols].reshape(KCH, BLK, UNION).transpose(1, 0, 2)
        b_maps.append(
            {
                "xT": np.ascontiguousarray(xu),
                "sqw": sq_full[cols].astype(np.float32),
                "tri": tri,
            }
        )
    nc_b = _get("main", _build_main_kernel)
    b_res = _run(nc_b, b_maps, "main")

    # ---- host combine (gather of partial sums only) ----
    E = S1 = S2 = FT = 0.0
    for c in range(NC_N):
        o = b_res[c]["out_stats"].astype(np.float64)
        E += o[:, 0].sum()
        S1 += o[:, 1].sum()
        S2 += o[:, 2].sum()
        FT += o[:, 3].sum()
    # device accumulates p = -u/2 and p^2 (with the 0.5-weighted region folded)
    S1 = -2.0 * S1
    S2 = 4.0 * S2

    N = float(N_PAIRS)
    mean_u = S1 / N
    mean = (mean_u + CC) / D
    var_u = (S2 - N * mean_u * mean_u) / (N - 1.0)
    std = np.sqrt(var_u) / D
    # logsumexp(-pdn/tau) = -CC*SS + log(E); loss = -that + log(N)
    loss = CC * SS - np.log(E) + np.log(N)
    feat_norm = FT / B

    return (
        np.float32(loss),
        np.float32(feat_norm),
        np.float32(mean),
        np.float32(std),
    )


if __name__ == "__main__":
    f = np.random.default_rng(0).standard_normal((B, 16, 768), dtype=np.float32)
    print(kernel(features=f))



# revision 19
# speedup vs baseline: 1.1004x; 1.1004x over previous
"""Trainium2 Bass kernel for nn_DispersiveLoss (B=2048, D=16*768=12288, 8 cores).

Single-launch circulant scheme:
  x (2048, 12288) -> 16 row-blocks of 128. Core c owns m-blocks {2c, 2c+1} and
  computes two Gram strips G[m, m..m+8 (mod 16)] (9 blocks = 1152 cols) in fp8
  (DoubleRow, D on partitions, PSUM fp32 accumulation). Own-block columns (xA)
  are DMA'd first so the two self-Gram blocks finish early; their diagonals
  g_ii = ||x_i||^2 are extracted on DVE, AllGathered across the 8 cores (256
  floats each), rotated into this core's 1280-col window via an indirect DMA
  driven by a per-core index input, and turned into centered bf16 corrections
  c = -(sq - D)/2.  A single K=2 bf16 matmul per segment adds c_i + c_j to
  PSUM, making p = -(d2 - 2D)/2 exactly as in the two-launch version.

  No triangle masks: the self blocks are symmetric, so the host subtracts the
  analytically known diagonal (p_ii = sq_i + 2 c_i) and halves. Stats per
  region (diag / full / half weights applied on host): ACT exp+accum for E,
  DVE copy+accum for S1, DVE square+accum for S2, staggered with the final
  matmul segments so the tail stays short.
"""

import os

import numpy as np
import ml_dtypes

import concourse.bass as bass
import concourse.mybir as mybir
import concourse.tile as tile
from concourse import bacc
from concourse.bass_utils import run_bass_kernel_spmd

NC_N = 8
B, D = 2048, 12288
BLK = 128
KCH = 96  # k-chunks of 128
TAU = 0.5
CC = float(2 * D)
S_EXP = 1.0 / (D * TAU)  # 1/6144
S2E = 2.0 * S_EXP  # ACT scale: exp(S2E * p) = exp(-u * S_EXP)
F32 = mybir.dt.float32
BF16 = mybir.dt.bfloat16
U32 = mybir.dt.uint32
DT_IN = mybir.dt.float8e4
NP_IN = ml_dtypes.float8_e4m3

N_PAIRS = B * (B - 1) // 2

KERNEL_EXEC_NS = []

_cache = {}

MULT = mybir.AluOpType.mult
ADD = mybir.AluOpType.add
EXP = mybir.ActivationFunctionType.Exp


def _trace_enabled():
    return bool(os.environ.get("KERNEL_TRACE"))


def _build_kernel():
    nc = bacc.Bacc("TRN2", target_bir_lowering=False, debug=False, num_devices=NC_N)
    xA = nc.dram_tensor("xA", [BLK, KCH, 256], DT_IN, kind="ExternalInput")
    xB = nc.dram_tensor("xB", [BLK, KCH, 1024], DT_IN, kind="ExternalInput")
    ident = nc.dram_tensor("ident", [BLK, BLK], F32, kind="ExternalInput")
    idx = nc.dram_tensor("idx", [16, 1], U32, kind="ExternalInput")
    out_acc = nc.dram_tensor("out_acc", [BLK, 18], F32, kind="ExternalOutput")
    out_sq = nc.dram_tensor("out_sq", [B], F32, kind="ExternalOutput")
    out_crow = nc.dram_tensor("out_crow", [1, 1280], F32, kind="ExternalOutput")

    DR = mybir.MatmulPerfMode.DoubleRow
    NPC = 12  # xA DMA pieces
    NGRP = 12  # xB DMA groups
    GC = KCH // NGRP  # 8 chunks per group

    with tile.TileContext(nc) as tc:
        with (
            tc.tile_pool(name="xa", bufs=12) as xap,
            tc.tile_pool(name="slab", bufs=4) as slab_pool,
            tc.tile_pool(name="ps", bufs=1, space="PSUM") as psp,
            tc.tile_pool(name="post", bufs=2) as post,
            tc.tile_pool(name="keep", bufs=1) as keep,
            tc.tile_pool(name="dram", bufs=1, space="DRAM") as drp,
        ):
            ps0 = psp.tile([BLK, 1152], F32, tag="ps0")
            ps1 = psp.tile([BLK, 1152], F32, tag="ps1")
            ps = [ps0, ps1]
            acc = keep.tile([BLK, 18], F32)

            # --- t0 setup: PE warm-up, exp-table preload, constant rows ---
            warm = keep.tile([BLK, 512], DT_IN, tag="warm")
            nc.gpsimd.memset(warm[:], 0.0)
            wps = psp.tile([BLK, 512], F32, tag="wps")
            NWARM = int(os.environ.get("KERNEL_NWARM", "4"))
            for _ in range(NWARM):
                nc.tensor.matmul(
                    wps[:], warm[:, 0:128], warm[:], start=True, stop=True,
                    skip_group_check=True,
                )
            # sacrificial start=True zero-matmuls: clear every PSUM region up
            # front (also warms the PE); all real matmuls accumulate with
            # start=False so a dropped first-accumulation can't lose data
            z2 = keep.tile([BLK, 2, 512], DT_IN, tag="z2")
            nc.gpsimd.memset(z2[:], 0.0)
            all_regions = [
                (0, 0, 128), (0, 128, 256), (0, 256, 512), (0, 512, 1024),
                (0, 1024, 1152),
                (1, 0, 128), (1, 128, 512), (1, 512, 1024), (1, 1024, 1152),
            ]
            for s, a, b in all_regions:
                nc.tensor.matmul(
                    ps[s][:, a:b], z2[:, :, 0:128], z2[:, :, 0 : b - a],
                    start=True, stop=False, perf_mode=DR,
                )
            zcol = keep.tile([BLK, 1], F32, tag="zcol")
            nc.gpsimd.memset(zcol[:], 0.0)
            zscr = keep.tile([BLK, 1], F32, tag="zscr")
            nc.scalar.activation(zscr[:], zcol[:], EXP)

            crow2 = keep.tile([2, 1280], BF16, tag="crow2")
            nc.gpsimd.memset(crow2[0:2, :], 1.0)
            lw0 = keep.tile([2, BLK], BF16, tag="lw0")
            lw1 = keep.tile([2, BLK], BF16, tag="lw1")
            nc.gpsimd.memset(lw0[0:2, :], 1.0)
            nc.gpsimd.memset(lw1[0:2, :], 1.0)

            ident_t = keep.tile([BLK, BLK], F32, tag="ident")
            nc.sync.dma_start(ident_t[:], ident[:])
            it = keep.tile([16, 1], U32, tag="idx")
            nc.sync.dma_start(it[:], idx[:])

            # --- DMA schedule (sync queue): xB g0 first, then xA, then rest ---
            xb_tiles = [None] * NGRP
            xb_tiles[0] = slab_pool.tile([BLK, GC, 1024], DT_IN, tag="slab", name="xb0")
            nc.sync.dma_start(xb_tiles[0][:], xB[:, 0:GC, :])
            PC = KCH // NPC
            xa_tiles = []
            for p in range(NPC):
                t = xap.tile([BLK, PC, 256], DT_IN, tag="xa", name=f"xa{p}")
                nc.sync.dma_start(t[:], xA[:, p * PC : (p + 1) * PC, :])
                xa_tiles.append(t)

            def xa_lhs(k, s):
                kk = k % PC
                return xa_tiles[k // PC][:, kk : kk + 2, 128 * s : 128 * s + 128]
            for g in range(1, NGRP):
                xb_tiles[g] = slab_pool.tile(
                    [BLK, GC, 1024], DT_IN, tag="slab", name=f"xb{g}"
                )
                nc.sync.dma_start(xb_tiles[g][:], xB[:, g * GC : (g + 1) * GC, :])

            # --- phase 1: xA-fed regions (diag0, ownpair, diag1) ---
            for p in range(NPC):
                for ii in range(0, PC, 2):
                    k = p * PC + ii
                    last = k == KCH - 2
                    lhs0 = xa_lhs(k, 0)
                    lhs1 = xa_lhs(k, 1)
                    nc.tensor.matmul(
                        ps0[:, 0:128], lhs0, lhs0,
                        start=False, stop=last, perf_mode=DR,
                    )
                    nc.tensor.matmul(
                        ps0[:, 128:256], lhs0, lhs1,
                        start=False, stop=False, perf_mode=DR,
                    )
                    nc.tensor.matmul(
                        ps1[:, 0:128], lhs1, lhs1,
                        start=False, stop=last, perf_mode=DR,
                    )

            # --- sq extraction + AllGather + rotation (no PE involvement) ---
            dcol = keep.tile([BLK, 2], F32, tag="dcol")
            for s in range(2):
                escr = post.tile([BLK, BLK], F32, tag="escr")
                nc.vector.scalar_tensor_tensor(
                    out=escr[:], in0=ps[s][:, 0:128], scalar=1.0, in1=ident_t[:],
                    op0=MULT, op1=MULT, accum_out=dcol[:, s : s + 1],
                )
            bounce = drp.tile([256], F32)
            obounce = drp.tile([B], F32)
            for s in range(2):
                nc.gpsimd.dma_start(
                    bounce[128 * s : 128 * s + 128].rearrange("(p o) -> p o", o=1),
                    dcol[:, s : s + 1],
                )
            nc.gpsimd.collective_compute(
                "AllGather",
                mybir.AluOpType.bypass,
                replica_groups=[list(range(NC_N))],
                ins=[bounce[:]],
                outs=[obounce[:]],
            )
            t10 = keep.tile([10, BLK], F32, tag="t10")
            nc.gpsimd.indirect_dma_start(
                out=t10[:],
                out_offset=None,
                in_=obounce[:].rearrange("(p f) -> p f", p=16),
                in_offset=bass.IndirectOffsetOnAxis(ap=it[0:10, 0:1], axis=0),
            )
            c10 = keep.tile([10, BLK], BF16, tag="c10")
            nc.vector.tensor_scalar(
                out=c10[:], in0=t10[:], scalar1=-float(D), scalar2=-0.5,
                op0=ADD, op1=MULT,
            )
            flatc = drp.tile([1280], BF16)
            nc.gpsimd.dma_start(flatc[:].rearrange("(p f) -> p f", p=10), c10[:])
            nc.gpsimd.dma_start(crow2[0:1, :], flatc[:].rearrange("(a b) -> a b", a=1))
            nc.gpsimd.dma_start(
                lw0[1:2, :], flatc[0:128].rearrange("(a b) -> a b", a=1)
            )
            nc.gpsimd.dma_start(
                lw1[1:2, :], flatc[128:256].rearrange("(a b) -> a b", a=1)
            )
            # out_sq for the host (diag corrections + feat_norm)
            ga = keep.tile([16, BLK], F32, tag="ga")
            nc.gpsimd.dma_start(ga[:], obounce[:].rearrange("(p f) -> p f", p=16))
            nc.gpsimd.dma_start(out_sq[:].rearrange("(p f) -> p f", p=16), ga[:])

            # --- phase 2: main Gram accumulation from xB ---
            # strip0 psum [256:1152] <- xB cols 0:896 ; strip1 psum [128:1152] <- xB 0:1024
            lws = [lw0, lw1]
            segs_main = [
                (0, (256, 512), (0, 256)),
                (0, (512, 1024), (256, 768)),
                (0, (1024, 1152), (768, 896)),
                (1, (128, 512), (0, 384)),
                (1, (512, 1024), (384, 896)),
                (1, (1024, 1152), (896, 1024)),
            ]
            for g in range(NGRP - 1):
                st = xb_tiles[g]
                for ii in range(0, GC, 2):
                    k = g * GC + ii
                    for s, (a, b), (ca, cb) in segs_main:
                        nc.tensor.matmul(
                            ps[s][:, a:b],
                            xa_lhs(k, s),
                            st[:, ii : ii + 2, ca:cb],
                            start=False, stop=False, perf_mode=DR,
                        )

            # --- final group: segment-major, stagger corrections + stats ---
            # stats regions per strip: diag [0:128] w=.5(sym), full [128:1024] w=1,
            # half [1024:1152] w=.5
            def corr(s, a, b):
                nc.tensor.matmul(
                    ps[s][:, a:b], lws[s][:], crow2[:, 128 * s + a : 128 * s + b],
                    start=False, stop=True, skip_group_check=True,
                )

            def stats(s, a, b, r):
                w = b - a
                e_s = post.tile([BLK, w], F32, tag=f"se{r}")
                nc.scalar.activation(
                    e_s[:], ps[s][:, a:b], EXP, scale=S2E,
                    accum_out=acc[:, 3 * s + r : 3 * s + r + 1],
                )
                c_s = post.tile([BLK, w], F32, tag=f"sc{r}")
                nc.vector.tensor_scalar(
                    out=c_s[:], in0=ps[s][:, a:b], scalar1=1.0, scalar2=0.0,
                    op0=MULT, op1=ADD,
                    accum_out=acc[:, 6 + 3 * s + r : 7 + 3 * s + r],
                )
                d_s = post.tile([BLK, w], F32, tag=f"sd{r}")
                nc.vector.scalar_tensor_tensor(
                    out=d_s[:], in0=c_s[:], scalar=1.0, in1=ps[s][:, a:b],
                    op0=MULT, op1=MULT,
                    accum_out=acc[:, 12 + 3 * s + r : 13 + 3 * s + r],
                )

            g = NGRP - 1
            st = xb_tiles[g]
            fsegs = {
                0: [(256, 512, 0, 256), (512, 1024, 256, 768), (1024, 1152, 768, 896)],
                1: [(128, 512, 0, 384), (512, 1024, 384, 896), (1024, 1152, 896, 1024)],
            }
            for s in range(2):
                for a, b, ca, cb in fsegs[s][:2]:
                    for ii in range(0, GC, 2):
                        nc.tensor.matmul(
                            ps[s][:, a:b],
                            xa_lhs(g * GC + ii, s),
                            st[:, ii : ii + 2, ca:cb],
                            start=False, stop=False, perf_mode=DR,
                        )
                # full region [128:1024] complete -> corrections + stats
                corr(s, 0, 512)
                corr(s, 512, 1024)
                stats(s, 0, 128, 0)
                stats(s, 128, 1024, 1)
                # half segment
                a, b, ca, cb = fsegs[s][2]
                for ii in range(0, GC, 2):
                    nc.tensor.matmul(
                        ps[s][:, a:b],
                        xa_lhs(g * GC + ii, s),
                        st[:, ii : ii + 2, ca:cb],
                        start=False, stop=False, perf_mode=DR,
                    )
                corr(s, 1024, 1152)
                stats(s, 1024, 1152, 2)

            nc.sync.dma_start(out_acc[:], acc[:])
            crow_f = keep.tile([1, 1280], F32, tag="crow_f")
            nc.vector.tensor_copy(crow_f[:], crow2[0:1, :])
            nc.sync.dma_start(out_crow[:], crow_f[:])
    nc.compile()
    return nc


def _get(name, builder):
    if name not in _cache:
        _cache[name] = builder()
    return _cache[name]


def _run(nc, in_maps, tag):
    if _trace_enabled():
        try:
            import profhook

            profhook.install()
        except Exception:
            pass
        import tempfile

        res = run_bass_kernel_spmd(
            nc, in_maps, list(range(NC_N)), trace=True,
            tmpdir=tempfile.mkdtemp(prefix=f"ktrace_{tag}_"),
        )
        KERNEL_EXEC_NS.append((tag, res.exec_time_ns))
        return res.results
    return run_bass_kernel_spmd(nc, in_maps, list(range(NC_N))).results


def kernel(features):
    x = np.asarray(features).reshape(B, D)
    xbf = x.astype(NP_IN)
    xT_full = np.ascontiguousarray(xbf.T)  # (D, B)

    ident = np.eye(BLK, dtype=np.float32)
    in_maps = []
    for c in range(NC_N):
        colsA = (256 * c + np.arange(256)) % B
        colsB = (256 * c + 256 + np.arange(1024)) % B
        xa = xT_full[:, colsA].reshape(KCH, BLK, 256).transpose(1, 0, 2)
        xb = xT_full[:, colsB].reshape(KCH, BLK, 1024).transpose(1, 0, 2)
        ix = np.array(
            [(2 * c + m) % 16 for m in range(16)], np.uint32
        ).reshape(16, 1)
        in_maps.append(
            {
                "xA": np.ascontiguousarray(xa),
                "xB": np.ascontiguousarray(xb),
                "ident": ident,
                "idx": ix,
            }
        )
    nc = _get("main", _build_kernel)
    res = _run(nc, in_maps, "main")

    # ---- host combine ----
    sq = res[0]["out_sq"].astype(np.float64)
    s2e = np.float64(np.float32(S2E))

    E = S1 = S2 = 0.0
    for c in range(NC_N):
        o = res[c]["out_acc"].astype(np.float64).sum(axis=0)  # [18]
        c_dev = res[c]["out_crow"].reshape(1280).astype(np.float64)
        for s in range(2):
            i0 = 256 * c + 128 * s
            p_d = sq[i0 : i0 + 128] + 2.0 * c_dev[128 * s : 128 * s + 128]
            E_dd = np.exp(s2e * p_d).sum()
            S1_dd = p_d.sum()
            S2_dd = (p_d * p_d).sum()
            E += o[3 * s + 1] + 0.5 * o[3 * s + 2] + 0.5 * (o[3 * s] - E_dd)
            S1 += (
                o[6 + 3 * s + 1]
                + 0.5 * o[6 + 3 * s + 2]
                + 0.5 * (o[6 + 3 * s] - S1_dd)
            )
            S2 += (
                o[12 + 3 * s + 1]
                + 0.5 * o[12 + 3 * s + 2]
                + 0.5 * (o[12 + 3 * s] - S2_dd)
            )

    sum_u = -2.0 * S1
    sum_u2 = 4.0 * S2
    N = float(N_PAIRS)
    mean_u = sum_u / N
    mean = (mean_u + CC) / D
    var_u = (sum_u2 - N * mean_u * mean_u) / (N - 1.0)
    std = np.sqrt(var_u) / D
    loss = CC * S_EXP - np.log(E) + np.log(N)
    feat_norm = np.sqrt(sq).mean()

    return (
        np.float32(loss),
        np.float32(feat_norm),
        np.float32(mean),
        np.float32(std),
    )


if __name__ == "__main__":
    f = np.random.default_rng(0).standard_normal((B, 16, 768), dtype=np.float32)
    print(kernel(features=f))


# revision 21
# speedup vs baseline: 1.3809x; 1.2549x over previous
"""Trainium2 Bass kernel for nn_DispersiveLoss (B=2048, D=16*768=12288, 8 cores).

Single-launch circulant scheme, no cross-core communication:
  x (2048, 12288) -> 16 row-blocks of 128. Core c owns m-blocks {2c, 2c+1} and
  computes two raw Gram strips G[m, m+1..m+8 (mod 16)] ([128,1024], fp8
  DoubleRow, D on partitions, fp32 PSUM) plus the two self blocks G[m,m] in a
  separate [128,256] PSUM tile.  The host performs the "gather": each core
  outputs its row norms (self-block diagonal), per-row region sums of g and
  g^2 (DVE accum), per-column sums of exp(S2E*g + bias_i) (ACT exp with local
  per-row bias c_i, reduced over rows by a ones-lhsT matmul), and per-column
  sums of g (gpsimd partition_all_reduce).  The host applies the per-column
  factors f_j = exp(S2E*c_j) and all linear c corrections in fp64, with
  c = -(sq - D)/2 so that u = d2 - 2D = -2*(g + c_i + c_j).

  Self blocks are symmetric: the host subtracts the analytically known
  diagonal and halves; distance-8 blocks (strip col 896:1024) are computed
  twice fleet-wide and weighted 0.5.  Every PSUM matmul stays inside a 2KB
  bank and every PSUM region gets a sacrificial start=True zero matmul up
  front (doubles as HAM warm-up); PSUM is strictly write-then-read.
"""

import os

import numpy as np
import ml_dtypes

import concourse.bass as bass
import concourse.bass_isa as bass_isa
import concourse.mybir as mybir
import concourse.tile as tile
from concourse import bacc
from concourse.bass_utils import run_bass_kernel_spmd

NC_N = 8
B, D = 2048, 12288
BLK = 128
KCH = 96
TAU = 0.5
CC = float(2 * D)
S_EXP = 1.0 / (D * TAU)
S2E = 2.0 * S_EXP
F32 = mybir.dt.float32
BF16 = mybir.dt.bfloat16
DT_IN = mybir.dt.float8e4
NP_IN = ml_dtypes.float8_e4m3

N_PAIRS = B * (B - 1) // 2

KERNEL_EXEC_NS = []

_cache = {}

MULT = mybir.AluOpType.mult
ADD = mybir.AluOpType.add
EXP = mybir.ActivationFunctionType.Exp

SEGS = [(0, 512), (512, 1024)]  # strip psum segments (bank-aligned)
REGS = [(0, 896), (896, 1024)]  # strip stats regions (full / half)


def _trace_enabled():
    return bool(os.environ.get("KERNEL_TRACE"))


def _build_kernel():
    nc = bacc.Bacc("TRN2", target_bir_lowering=False, debug=False, num_devices=NC_N)
    xT = nc.dram_tensor("xT", [BLK, KCH, 1280], DT_IN, kind="ExternalInput")
    ident = nc.dram_tensor("ident", [BLK, BLK], F32, kind="ExternalInput")
    zin = nc.dram_tensor("zin", [BLK, 2, 512], BF16, kind="ExternalInput")
    onesw = nc.dram_tensor("onesw", [BLK, 4], BF16, kind="ExternalInput")
    out_acc = nc.dram_tensor("out_acc", [BLK, 14], F32, kind="ExternalOutput")
    out_cols = nc.dram_tensor("out_cols", [4, 1152], F32, kind="ExternalOutput")

    DR = mybir.MatmulPerfMode.DoubleRow
    NGRP = 12
    GC = KCH // NGRP  # 8 chunks per DMA group

    with tile.TileContext(nc) as tc:
        with (
            tc.tile_pool(name="slab", bufs=4) as slab_pool,
            tc.tile_pool(name="ps", bufs=1, space="PSUM") as psp,
            tc.tile_pool(name="post", bufs=2) as post,
            tc.tile_pool(name="keep", bufs=1) as keep,
        ):
            ps0 = psp.tile([BLK, 1024], F32, tag="ps0")
            ps1 = psp.tile([BLK, 1024], F32, tag="ps1")
            pd = psp.tile([BLK, 256], F32, tag="pd")
            pcs = psp.tile([2, 1152], F32, tag="pcs")
            ps = [ps0, ps1]
            acc = keep.tile([BLK, 14], F32)

            # --- consts ---
            ident_t = keep.tile([BLK, BLK], F32, tag="ident")
            nc.sync.dma_start(ident_t[:], ident[:])
            z2 = keep.tile([BLK, 2, 512], BF16, tag="z2")
            nc.sync.dma_start(z2[:], zin[:])
            ow = keep.tile([BLK, 4], BF16, tag="ow")
            nc.sync.dma_start(ow[:], onesw[:])

            # --- sacrificial start=True zero matmuls (also HAM warm-up) ---
            for s in range(2):
                for a, b in SEGS:
                    nc.tensor.matmul(
                        ps[s][:, a:b], z2[:, 0, 0:128], z2[:, 0, 0 : b - a],
                        start=True, stop=False,
                    )
            for s in range(2):
                nc.tensor.matmul(
                    pd[:, 128 * s : 128 * s + 128], z2[:, 0, 0:128],
                    z2[:, 0, 0:128], start=True, stop=False,
                )
            for a, b in [(0, 512), (512, 1024), (1024, 1152)]:
                nc.tensor.matmul(
                    pcs[:, a:b], z2[:, 0, 0:2], z2[:, 0, 0 : b - a],
                    start=True, stop=False,
                )
            zscr = keep.tile([BLK, 1], F32, tag="zscr")
            nc.scalar.activation(zscr[:], ident_t[:, 1:2], EXP)

            # --- slab DMAs ---
            xb_tiles = []
            for g in range(NGRP):
                t = slab_pool.tile([BLK, GC, 1280], DT_IN, tag="slab", name=f"sl{g}")
                nc.sync.dma_start(t[:], xT[:, g * GC : (g + 1) * GC, :])
                xb_tiles.append(t)

            def kpair_mms(st, ii, s, last=False):
                lhs = st[:, ii : ii + 2, 128 * s : 128 * s + 128]
                for a, b in SEGS:
                    off = 128 * (s + 1)
                    nc.tensor.matmul(
                        ps[s][:, a:b], lhs,
                        st[:, ii : ii + 2, off + a : off + b],
                        start=False, stop=last, perf_mode=DR,
                    )
                nc.tensor.matmul(
                    pd[:, 128 * s : 128 * s + 128], lhs, lhs,
                    start=False, stop=last, perf_mode=DR,
                )

            # --- Gram accumulation (groups 0..10) ---
            for g in range(NGRP - 1):
                st = xb_tiles[g]
                for ii in range(0, GC, 2):
                    for s in range(2):
                        kpair_mms(st, ii, s)

            # --- final group: strip-major, stats staggered ---
            st = xb_tiles[NGRP - 1]
            et = []
            etd = keep.tile([BLK, 256], BF16, tag="etd")
            bt = []
            bd = keep.tile([BLK, 256], BF16, tag="bd")
            for s in range(2):
                for ii in range(0, GC, 2):
                    kpair_mms(st, ii, s, last=(ii == GC - 2))
                # self-block diagonal -> sq (acc col 12+s)
                escr = post.tile([BLK, BLK], F32, tag="escr")
                nc.vector.scalar_tensor_tensor(
                    out=escr[:], in0=pd[:, 128 * s : 128 * s + 128], scalar=1.0,
                    in1=ident_t[:], op0=MULT, op1=MULT,
                    accum_out=acc[:, 12 + s : 13 + s],
                )
                # local row bias: S2E * c_i = (sq_i - D) * (-0.5 * S2E)
                bias_s = keep.tile([BLK, 1], F32, tag=f"bias{s}", name=f"bias{s}")
                nc.vector.tensor_scalar(
                    out=bias_s[:], in0=acc[:, 12 + s : 13 + s],
                    scalar1=-float(D), scalar2=-0.5 * S2E, op0=ADD, op1=MULT,
                )
                # et = exp(S2E*g + bias_i) in bf16
                et_s = keep.tile([BLK, 1024], BF16, tag=f"et{s}", name=f"et{s}")
                nc.scalar.activation(
                    et_s[:], ps[s][:, 0:1024], EXP, scale=S2E, bias=bias_s[:]
                )
                et.append(et_s)
                nc.scalar.activation(
                    etd[:, 128 * s : 128 * s + 128],
                    pd[:, 128 * s : 128 * s + 128], EXP, scale=S2E, bias=bias_s[:],
                )
                # b = bf16 copy of g with per-region per-row S1 accums
                b_s = keep.tile([BLK, 1024], BF16, tag=f"b{s}", name=f"b{s}")
                for r, (a, b) in enumerate(REGS):
                    nc.vector.tensor_scalar(
                        out=b_s[:, a:b], in0=ps[s][:, a:b], scalar1=1.0,
                        scalar2=0.0, op0=MULT, op1=ADD,
                        accum_out=acc[:, 2 * s + r : 2 * s + r + 1],
                    )
                bt.append(b_s)
                nc.vector.tensor_scalar(
                    out=bd[:, 128 * s : 128 * s + 128],
                    in0=pd[:, 128 * s : 128 * s + 128], scalar1=1.0,
                    scalar2=0.0, op0=MULT, op1=ADD,
                    accum_out=acc[:, 4 + s : 5 + s],
                )
                # S2 row sums: sum b*g per region
                for r, (a, b) in enumerate(REGS):
                    scr = post.tile([BLK, b - a], F32, tag=f"scr{r}")
                    nc.vector.scalar_tensor_tensor(
                        out=scr[:], in0=b_s[:, a:b], scalar=1.0, in1=ps[s][:, a:b],
                        op0=MULT, op1=MULT,
                        accum_out=acc[:, 6 + 2 * s + r : 7 + 2 * s + r],
                    )
                scrd = post.tile([BLK, BLK], F32, tag="scrd")
                nc.vector.scalar_tensor_tensor(
                    out=scrd[:], in0=bd[:, 128 * s : 128 * s + 128], scalar=1.0,
                    in1=pd[:, 128 * s : 128 * s + 128], op0=MULT, op1=MULT,
                    accum_out=acc[:, 10 + s : 11 + s],
                )

            # --- column sums of et via ones-lhsT matmuls into pcs rows ---
            for s in range(2):
                for a, b in SEGS:
                    nc.tensor.matmul(
                        pcs[:, a:b], ow[:, 2 * s : 2 * s + 2], et[s][:, a:b],
                        start=False, stop=False,
                    )
                nc.tensor.matmul(
                    pcs[:, 1024:1152], ow[:, 2 * s : 2 * s + 2],
                    etd[:, 128 * s : 128 * s + 128],
                    start=False, stop=(s == 1),
                )
            cse = keep.tile([2, 1152], F32, tag="cse")
            nc.vector.tensor_copy(cse[:], pcs[:])

            # --- column sums of g via gpsimd partition all-reduce ---
            prr = []
            for s in range(2):
                pr_s = keep.tile([BLK, 1024], F32, tag=f"pr{s}", name=f"pr{s}")
                nc.gpsimd.partition_all_reduce(
                    pr_s[:], bt[s][:], channels=BLK, reduce_op=bass_isa.ReduceOp.add
                )
                prr.append(pr_s)
            prd = keep.tile([BLK, 256], F32, tag="prd")
            nc.gpsimd.partition_all_reduce(
                prd[:], bd[:], channels=BLK, reduce_op=bass_isa.ReduceOp.add
            )

            nc.sync.dma_start(out_acc[:], acc[:])
            nc.sync.dma_start(out_cols[0:2, :], cse[:])
            for s in range(2):
                nc.sync.dma_start(
                    out_cols[2 + s : 3 + s, 0:1024], prr[s][0:1, :]
                )
                nc.sync.dma_start(
                    out_cols[2 + s : 3 + s, 1024:1152],
                    prd[0:1, 128 * s : 128 * s + 128],
                )
    nc.compile()
    return nc


def _get(name, builder):
    if name not in _cache:
        _cache[name] = builder()
    return _cache[name]


def _run(nc, in_maps, tag):
    if _trace_enabled():
        try:
            import profhook

            profhook.install()
        except Exception:
            pass
        import tempfile

        res = run_bass_kernel_spmd(
            nc, in_maps, list(range(NC_N)), trace=True,
            tmpdir=tempfile.mkdtemp(prefix=f"ktrace_{tag}_"),
        )
        KERNEL_EXEC_NS.append((tag, res.exec_time_ns))
        return res.results
    return run_bass_kernel_spmd(nc, in_maps, list(range(NC_N))).results


def kernel(features):
    x = np.asarray(features).reshape(B, D)
    xbf = x.astype(NP_IN)
    xT_full = np.ascontiguousarray(xbf.T)  # (D, B)

    ident = np.eye(BLK, dtype=np.float32)
    zin = np.zeros((BLK, 2, 512), dtype=ml_dtypes.bfloat16)
    onesw = np.zeros((BLK, 4), dtype=ml_dtypes.bfloat16)
    onesw[:, 0] = 1.0  # strip0 lhsT -> psum row 0
    onesw[:, 3] = 1.0  # strip1 lhsT -> psum row 1
    in_maps = []
    for c in range(NC_N):
        cols = (256 * c + np.arange(1280)) % B
        xu = xT_full[:, cols].reshape(KCH, BLK, 1280).transpose(1, 0, 2)
        in_maps.append(
            {
                "xT": np.ascontiguousarray(xu),
                "ident": ident,
                "zin": zin,
                "onesw": onesw,
            }
        )
    nc = _get("main", _build_kernel)
    res = _run(nc, in_maps, "main")

    # ---- host combine (fp64) ----
    sq = np.zeros(B)
    for c in range(NC_N):
        a = res[c]["out_acc"].astype(np.float64)
        sq[256 * c : 256 * c + 128] = a[:, 12]
        sq[256 * c + 128 : 256 * c + 256] = a[:, 13]
    cvec = -(sq - D) / 2.0
    s2e = np.float64(np.float32(S2E))
    fvec = np.exp(s2e * cvec)
    sq_b16 = sq.astype(np.float32).astype(ml_dtypes.bfloat16).astype(np.float64)

    E = S1 = S2 = 0.0
    for c in range(NC_N):
        A = res[c]["out_acc"].astype(np.float64)
        CS = res[c]["out_cols"].astype(np.float64)
        for s in range(2):
            rows = 256 * c + 128 * s + np.arange(128)
            scols = (256 * c + 128 * (s + 1) + np.arange(1024)) % B
            c_row = cvec[rows]
            # strip regions: full (w=1), half (w=0.5)
            for r, (a, b) in enumerate(REGS):
                W = b - a
                cR = cvec[scols[a:b]]
                s1rows = A[:, 2 * s + r]
                S1g = s1rows.sum()
                S2g = A[:, 6 + 2 * s + r].sum()
                E_reg = (fvec[scols[a:b]] * CS[s, a:b]).sum()
                S1_reg = S1g + W * c_row.sum() + 128.0 * cR.sum()
                S2_reg = (
                    S2g
                    + 2.0 * (c_row * s1rows).sum()
                    + 2.0 * (cR * CS[2 + s, a:b]).sum()
                    + W * (c_row**2).sum()
                    + 2.0 * c_row.sum() * cR.sum()
                    + 128.0 * (cR**2).sum()
                )
                w = 1.0 if r == 0 else 0.5
                E += w * E_reg
                S1 += w * S1_reg
                S2 += w * S2_reg
            # self block: remove diagonal, halve
            cR = c_row
            s1rows = A[:, 4 + s]
            S1g = s1rows.sum()
            S2g = A[:, 10 + s].sum()
            E_reg = (fvec[rows] * CS[s, 1024:1152]).sum()
            S1_reg = S1g + 128.0 * c_row.sum() + 128.0 * cR.sum()
            S2_reg = (
                S2g
                + 2.0 * (c_row * s1rows).sum()
                + 2.0 * (cR * CS[2 + s, 1024:1152]).sum()
                + 128.0 * (c_row**2).sum()
                + 2.0 * c_row.sum() * cR.sum()
                + 128.0 * (cR**2).sum()
            )
            sqr = sq[rows]
            bqr = sq_b16[rows]
            E_dd = (fvec[rows] * np.exp(s2e * (sqr + c_row))).sum()
            S1_dd = (bqr + 2.0 * c_row).sum()
            S2_dd = (bqr * sqr + 4.0 * c_row * bqr + 4.0 * c_row**2).sum()
            E += 0.5 * (E_reg - E_dd)
            S1 += 0.5 * (S1_reg - S1_dd)
            S2 += 0.5 * (S2_reg - S2_dd)

    sum_u = -2.0 * S1
    sum_u2 = 4.0 * S2
    N = float(N_PAIRS)
    mean_u = sum_u / N
    mean = (mean_u + CC) / D
    var_u = (sum_u2 - N * mean_u * mean_u) / (N - 1.0)
    std = np.sqrt(var_u) / D
    loss = CC * S_EXP - np.log(E) + np.log(N)
    feat_norm = np.sqrt(sq).mean()

    return (
        np.float32(loss),
        np.float32(feat_norm),
        np.float32(mean),
        np.float32(std),
    )


if __name__ == "__main__":
    f = np.random.default_rng(0).standard_normal((B, 16, 768), dtype=np.float32)
    print(kernel(features=f))


# revision 22
# speedup vs baseline: 1.4608x; 1.0579x over previous
"""Trainium2 Bass kernel for nn_DispersiveLoss (B=2048, D=16*768=12288, 8 cores).

Single-launch circulant scheme, no cross-core communication:
  x (2048, 12288) -> 16 row-blocks of 128. Core c owns m-blocks {2c, 2c+1} and
  computes two raw Gram strips G[m, m+1..m+8 (mod 16)] ([128,1024], fp8
  DoubleRow, D on partitions, fp32 PSUM) plus the two self blocks G[m,m] in a
  separate [128,256] PSUM tile.  The host performs the "gather": each core
  outputs its row norms (self-block diagonal), per-row region sums of g and
  g^2 (DVE accum), per-column sums of exp(S2E*g + bias_i) (ACT exp with local
  per-row bias c_i, reduced over rows by a ones-lhsT matmul), and per-column
  sums of g (gpsimd partition_all_reduce).  The host applies the per-column
  factors f_j = exp(S2E*c_j) and all linear c corrections in fp64, with
  c = -(sq - D)/2 so that u = d2 - 2D = -2*(g + c_i + c_j).

  Self blocks are symmetric: the host subtracts the analytically known
  diagonal and halves; distance-8 blocks (strip col 896:1024) are computed
  twice fleet-wide and weighted 0.5.  Every PSUM matmul stays inside a 2KB
  bank and every PSUM region gets a sacrificial start=True zero matmul up
  front (doubles as HAM warm-up); PSUM is strictly write-then-read.
"""

import os

import numpy as np
import ml_dtypes

import concourse.bass as bass
import concourse.bass_isa as bass_isa
import concourse.mybir as mybir
import concourse.tile as tile
from concourse import bacc
from concourse.bass_utils import run_bass_kernel_spmd

NC_N = 8
B, D = 2048, 12288
BLK = 128
KCH = 96
TAU = 0.5
CC = float(2 * D)
S_EXP = 1.0 / (D * TAU)
S2E = 2.0 * S_EXP
F32 = mybir.dt.float32
BF16 = mybir.dt.bfloat16
DT_IN = mybir.dt.float8e4
NP_IN = ml_dtypes.float8_e4m3

N_PAIRS = B * (B - 1) // 2

KERNEL_EXEC_NS = []

_cache = {}

MULT = mybir.AluOpType.mult
ADD = mybir.AluOpType.add
EXP = mybir.ActivationFunctionType.Exp

SEGS = [(0, 512), (512, 1024)]  # strip psum segments (bank-aligned)
REGS = [(0, 896), (896, 1024)]  # strip stats regions (full / half)


def _trace_enabled():
    return bool(os.environ.get("KERNEL_TRACE"))


def _build_kernel():
    nc = bacc.Bacc("TRN2", target_bir_lowering=False, debug=False, num_devices=NC_N)
    xT = nc.dram_tensor("xT", [BLK, KCH, 1280], DT_IN, kind="ExternalInput")
    ident = nc.dram_tensor("ident", [BLK, BLK], F32, kind="ExternalInput")
    zin = nc.dram_tensor("zin", [BLK, 2, 512], DT_IN, kind="ExternalInput")
    onesw = nc.dram_tensor("onesw", [BLK, 4], BF16, kind="ExternalInput")
    out_acc = nc.dram_tensor("out_acc", [BLK, 14], F32, kind="ExternalOutput")
    out_cols = nc.dram_tensor("out_cols", [4, 1152], F32, kind="ExternalOutput")

    DR = mybir.MatmulPerfMode.DoubleRow
    NGRP = 12
    GC = KCH // NGRP  # 8 chunks per DMA group

    with tile.TileContext(nc) as tc:
        with (
            tc.tile_pool(name="slab", bufs=4) as slab_pool,
            tc.tile_pool(name="ps", bufs=1, space="PSUM") as psp,
            tc.tile_pool(name="post", bufs=2) as post,
            tc.tile_pool(name="keep", bufs=1) as keep,
        ):
            ps0 = psp.tile([BLK, 1024], F32, tag="ps0")
            ps1 = psp.tile([BLK, 1024], F32, tag="ps1")
            pd = psp.tile([BLK, 256], F32, tag="pd")
            pcs = psp.tile([2, 1152], F32, tag="pcs")
            ps = [ps0, ps1]
            acc = keep.tile([BLK, 14], F32)

            # --- consts (z2 first; ident/ow issued mid-stream) ---
            z2 = keep.tile([BLK, 2, 512], DT_IN, tag="z2")
            nc.sync.dma_start(z2[:], zin[:])
            ident_t = keep.tile([BLK, BLK], F32, tag="ident")
            ow = keep.tile([BLK, 4], BF16, tag="ow")

            # --- sacrificial start=True zero matmuls (also HAM warm-up) ---
            for s in range(2):
                for a, b in SEGS:
                    nc.tensor.matmul(
                        ps[s][:, a:b], z2[:, 0, 0:128], z2[:, 0, 0 : b - a],
                        start=True, stop=False,
                    )
            for s in range(2):
                nc.tensor.matmul(
                    pd[:, 128 * s : 128 * s + 128], z2[:, 0, 0:128],
                    z2[:, 0, 0:128], start=True, stop=False,
                )
            for a, b in [(0, 512), (512, 1024), (1024, 1152)]:
                nc.tensor.matmul(
                    pcs[:, a:b], z2[:, 0, 0:2], z2[:, 0, 0 : b - a],
                    start=True, stop=False,
                )
            zscr = keep.tile([BLK, 1], F32, tag="zscr")
            nc.scalar.activation(zscr[:], z2[:, 0, 0:1], EXP)

            # --- slab DMAs (alternate between SP and ACT DGE queues) ---
            xb_tiles = []
            for g in range(NGRP):
                t = slab_pool.tile([BLK, GC, 1280], DT_IN, tag="slab", name=f"sl{g}")
                eng = nc.sync if g % 2 == 0 else nc.scalar
                eng.dma_start(t[:], xT[:, g * GC : (g + 1) * GC, :])
                xb_tiles.append(t)
                if g == 6:
                    nc.sync.dma_start(ident_t[:], ident[:])
                    nc.sync.dma_start(ow[:], onesw[:])

            def kpair_mms(st, ii, s, last=False):
                lhs = st[:, ii : ii + 2, 128 * s : 128 * s + 128]
                for a, b in SEGS:
                    off = 128 * (s + 1)
                    nc.tensor.matmul(
                        ps[s][:, a:b], lhs,
                        st[:, ii : ii + 2, off + a : off + b],
                        start=False, stop=last, perf_mode=DR,
                    )
                nc.tensor.matmul(
                    pd[:, 128 * s : 128 * s + 128], lhs, lhs,
                    start=False, stop=last, perf_mode=DR,
                )

            # --- Gram accumulation (groups 0..10) ---
            for g in range(NGRP - 1):
                st = xb_tiles[g]
                for ii in range(0, GC, 2):
                    for s in range(2):
                        kpair_mms(st, ii, s)

            # --- final group: strip-major, stats staggered ---
            st = xb_tiles[NGRP - 1]
            et = []
            etd = keep.tile([BLK, 256], BF16, tag="etd")
            bt = []
            bd = keep.tile([BLK, 256], BF16, tag="bd")
            for s in range(2):
                for ii in range(0, GC, 2):
                    kpair_mms(st, ii, s, last=(ii == GC - 2))
                # self-block diagonal -> sq (acc col 12+s)
                escr = post.tile([BLK, BLK], F32, tag="escr")
                nc.vector.scalar_tensor_tensor(
                    out=escr[:], in0=pd[:, 128 * s : 128 * s + 128], scalar=1.0,
                    in1=ident_t[:], op0=MULT, op1=MULT,
                    accum_out=acc[:, 12 + s : 13 + s],
                )
                # local row bias: S2E * c_i = (sq_i - D) * (-0.5 * S2E)
                bias_s = keep.tile([BLK, 1], F32, tag=f"bias{s}", name=f"bias{s}")
                nc.vector.tensor_scalar(
                    out=bias_s[:], in0=acc[:, 12 + s : 13 + s],
                    scalar1=-float(D), scalar2=-0.5 * S2E, op0=ADD, op1=MULT,
                )
                # et = exp(S2E*g + bias_i) in bf16
                et_s = keep.tile([BLK, 1024], BF16, tag=f"et{s}", name=f"et{s}")
                nc.scalar.activation(
                    et_s[:], ps[s][:, 0:1024], EXP, scale=S2E, bias=bias_s[:]
                )
                et.append(et_s)
                nc.scalar.activation(
                    etd[:, 128 * s : 128 * s + 128],
                    pd[:, 128 * s : 128 * s + 128], EXP, scale=S2E, bias=bias_s[:],
                )
                # b = bf16 copy of g with per-region per-row S1 accums
                b_s = keep.tile([BLK, 1024], BF16, tag=f"b{s}", name=f"b{s}")
                for r, (a, b) in enumerate(REGS):
                    nc.vector.tensor_scalar(
                        out=b_s[:, a:b], in0=ps[s][:, a:b], scalar1=1.0,
                        scalar2=0.0, op0=MULT, op1=ADD,
                        accum_out=acc[:, 2 * s + r : 2 * s + r + 1],
                    )
                bt.append(b_s)
                nc.vector.tensor_scalar(
                    out=bd[:, 128 * s : 128 * s + 128],
                    in0=pd[:, 128 * s : 128 * s + 128], scalar1=1.0,
                    scalar2=0.0, op0=MULT, op1=ADD,
                    accum_out=acc[:, 4 + s : 5 + s],
                )
                # S2 row sums: sum b*g per region
                for r, (a, b) in enumerate(REGS):
                    scr = post.tile([BLK, b - a], F32, tag=f"scr{r}")
                    nc.vector.scalar_tensor_tensor(
                        out=scr[:], in0=b_s[:, a:b], scalar=1.0, in1=ps[s][:, a:b],
                        op0=MULT, op1=MULT,
                        accum_out=acc[:, 6 + 2 * s + r : 7 + 2 * s + r],
                    )
                scrd = post.tile([BLK, BLK], F32, tag="scrd")
                nc.vector.scalar_tensor_tensor(
                    out=scrd[:], in0=bd[:, 128 * s : 128 * s + 128], scalar=1.0,
                    in1=pd[:, 128 * s : 128 * s + 128], op0=MULT, op1=MULT,
                    accum_out=acc[:, 10 + s : 11 + s],
                )

            # --- column sums of et via ones-lhsT matmuls into pcs rows ---
            for s in range(2):
                for a, b in SEGS:
                    nc.tensor.matmul(
                        pcs[:, a:b], ow[:, 2 * s : 2 * s + 2], et[s][:, a:b],
                        start=False, stop=False,
                    )
                nc.tensor.matmul(
                    pcs[:, 1024:1152], ow[:, 2 * s : 2 * s + 2],
                    etd[:, 128 * s : 128 * s + 128],
                    start=False, stop=(s == 1),
                )
            cse = keep.tile([2, 1152], F32, tag="cse")
            nc.vector.tensor_copy(cse[:], pcs[:])

            # --- column sums of g: clear pcs, reuse it on the PE ---
            for a, b in [(0, 512), (512, 1024), (1024, 1152)]:
                nc.tensor.matmul(
                    pcs[:, a:b], z2[:, 0, 0:2], z2[:, 0, 0 : b - a],
                    start=True, stop=False, skip_group_check=True,
                )
            for s in range(2):
                for a, b in SEGS:
                    nc.tensor.matmul(
                        pcs[:, a:b], ow[:, 2 * s : 2 * s + 2], bt[s][:, a:b],
                        start=False, stop=False,
                    )
                nc.tensor.matmul(
                    pcs[:, 1024:1152], ow[:, 2 * s : 2 * s + 2],
                    bd[:, 128 * s : 128 * s + 128],
                    start=False, stop=(s == 1),
                )
            csg = keep.tile([2, 1152], F32, tag="csg")
            nc.vector.tensor_copy(csg[:], pcs[:])

            nc.sync.dma_start(out_acc[:], acc[:])
            nc.sync.dma_start(out_cols[0:2, :], cse[:])
            nc.sync.dma_start(out_cols[2:4, :], csg[:])
    nc.compile()
    return nc


def _get(name, builder):
    if name not in _cache:
        _cache[name] = builder()
    return _cache[name]


def _run(nc, in_maps, tag):
    if _trace_enabled():
        try:
            import profhook

            profhook.install()
        except Exception:
            pass
        import tempfile

        res = run_bass_kernel_spmd(
            nc, in_maps, list(range(NC_N)), trace=True,
            tmpdir=tempfile.mkdtemp(prefix=f"ktrace_{tag}_"),
        )
        KERNEL_EXEC_NS.append((tag, res.exec_time_ns))
        return res.results
    return run_bass_kernel_spmd(nc, in_maps, list(range(NC_N))).results


def kernel(features):
    x = np.asarray(features).reshape(B, D)
    xbf = x.astype(NP_IN)
    xT_full = np.ascontiguousarray(xbf.T)  # (D, B)

    ident = np.eye(BLK, dtype=np.float32)
    zin = np.zeros((BLK, 2, 512), dtype=NP_IN)
    onesw = np.zeros((BLK, 4), dtype=ml_dtypes.bfloat16)
    onesw[:, 0] = 1.0  # strip0 lhsT -> psum row 0
    onesw[:, 3] = 1.0  # strip1 lhsT -> psum row 1
    in_maps = []
    for c in range(NC_N):
        cols = (256 * c + np.arange(1280)) % B
        xu = xT_full[:, cols].reshape(KCH, BLK, 1280).transpose(1, 0, 2)
        in_maps.append(
            {
                "xT": np.ascontiguousarray(xu),
                "ident": ident,
                "zin": zin,
                "onesw": onesw,
            }
        )
    nc = _get("main", _build_kernel)
    res = _run(nc, in_maps, "main")

    # ---- host combine (fp64) ----
    sq = np.zeros(B)
    for c in range(NC_N):
        a = res[c]["out_acc"].astype(np.float64)
        sq[256 * c : 256 * c + 128] = a[:, 12]
        sq[256 * c + 128 : 256 * c + 256] = a[:, 13]
    cvec = -(sq - D) / 2.0
    s2e = np.float64(np.float32(S2E))
    fvec = np.exp(s2e * cvec)
    sq_b16 = sq.astype(np.float32).astype(ml_dtypes.bfloat16).astype(np.float64)

    E = S1 = S2 = 0.0
    for c in range(NC_N):
        A = res[c]["out_acc"].astype(np.float64)
        CS = res[c]["out_cols"].astype(np.float64)
        for s in range(2):
            rows = 256 * c + 128 * s + np.arange(128)
            scols = (256 * c + 128 * (s + 1) + np.arange(1024)) % B
            c_row = cvec[rows]
            # strip regions: full (w=1), half (w=0.5)
            for r, (a, b) in enumerate(REGS):
                W = b - a
                cR = cvec[scols[a:b]]
                s1rows = A[:, 2 * s + r]
                S1g = s1rows.sum()
                S2g = A[:, 6 + 2 * s + r].sum()
                E_reg = (fvec[scols[a:b]] * CS[s, a:b]).sum()
                S1_reg = S1g + W * c_row.sum() + 128.0 * cR.sum()
                S2_reg = (
                    S2g
                    + 2.0 * (c_row * s1rows).sum()
                    + 2.0 * (cR * CS[2 + s, a:b]).sum()
                    + W * (c_row**2).sum()
                    + 2.0 * c_row.sum() * cR.sum()
                    + 128.0 * (cR**2).sum()
                )
                w = 1.0 if r == 0 else 0.5
                E += w * E_reg
                S1 += w * S1_reg
                S2 += w * S2_reg
            # self block: remove diagonal, halve
            cR = c_row
            s1rows = A[:, 4 + s]
            S1g = s1rows.sum()
            S2g = A[:, 10 + s].sum()
            E_reg = (fvec[rows] * CS[s, 1024:1152]).sum()
            S1_reg = S1g + 128.0 * c_row.sum() + 128.0 * cR.sum()
            S2_reg = (
                S2g
                + 2.0 * (c_row * s1rows).sum()
                + 2.0 * (cR * CS[2 + s, 1024:1152]).sum()
                + 128.0 * (c_row**2).sum()
                + 2.0 * c_row.sum() * cR.sum()
                + 128.0 * (cR**2).sum()
            )
            sqr = sq[rows]
            bqr = sq_b16[rows]
            E_dd = (fvec[rows] * np.exp(s2e * (sqr + c_row))).sum()
            S1_dd = (bqr + 2.0 * c_row).sum()
            S2_dd = (bqr * sqr + 4.0 * c_row * bqr + 4.0 * c_row**2).sum()
            E += 0.5 * (E_reg - E_dd)
            S1 += 0.5 * (S1_reg - S1_dd)
            S2 += 0.5 * (S2_reg - S2_dd)

    sum_u = -2.0 * S1
    sum_u2 = 4.0 * S2
    N = float(N_PAIRS)
    mean_u = sum_u / N
    mean = (mean_u + CC) / D
    var_u = (sum_u2 - N * mean_u * mean_u) / (N - 1.0)
    std = np.sqrt(var_u) / D
    loss = CC * S_EXP - np.log(E) + np.log(N)
    feat_norm = np.sqrt(sq).mean()

    return (
        np.float32(loss),
        np.float32(feat_norm),
        np.float32(mean),
        np.float32(std),
    )


if __name__ == "__main__":
    f = np.random.default_rng(0).standard_normal((B, 16, 768), dtype=np.float32)
    print(kernel(features=f))
